# revision 1
# baseline (speedup 1.0000x reference)
"""Trainium2 Bass kernel for nn_Attention_78151224918608.

Dense transformer attention block: QKV proj + RoPE + GQA causal attention
+ output proj. Sharding: tensor-parallel over heads across 8 cores
(core c: Q heads 4c..4c+3, KV head c). Each core computes a partial
output (its heads through wo rows); host sums the 8 bf16 partials in
fp32 and casts to bf16.

Layout strategy (per core, per batch):
  - All matmul operands bf16; accumulation fp32 in PSUM.
  - Projections computed transposed: QKV^T[384, S] = wqkv^T @ x^T so that
    Q^T/K^T (head-dim on partitions) feed the scores matmul directly.
  - RoPE: even/odd pair interleave is folded into wq/wk/wo columns on the
    host (perm = evens-then-odds), turning the pair swap into a 32-row
    block swap done with a small permutation matmul on PE.
  - Scores computed transposed per (b,h): S^T[k,q] = K^T.T @ Q^T, so the
    softmax denominator and P@V both contract over k = partitions:
    PV lhsT = [V | ones-col] gives O^T rows 0:64 and sumexp in row 64.
  - Causal: only k-tiles <= q-tile are computed; diagonal 128x128 blocks
    get an additive triangular mask in PSUM before exp; fully-invalid
    column strips of the exp tile are memset to 0 afterwards.
  - exp on ScalarE reads PSUM strips [128, 1024] and writes bf16 SBUF.
  - Normalization: recip = 1/sumexp (DVE), broadcast across 64 partitions
    with a K=1 ones matmul, multiplied into O^T during evacuation.
"""

import sys

sys.path.insert(0, "/opt/trn_rl_repo")

import math
import numpy as np
import ml_dtypes

BF16 = ml_dtypes.bfloat16

# Problem constants (hardcoded per contract).
B = 2
S = 2048
D = 2048
N_HEADS = 32
N_KV_HEADS = 8
HD = 64
N_CORES = 8
HQ = N_HEADS // N_CORES  # 4 q heads per core
M_PROJ = HQ * HD + 2 * HD  # 384: [Q0 Q1 Q2 Q3 | K | V]
QTS = 512  # q tile size (free dim)
KTS = 128  # k tile size (partitions)
GRP = 2  # k-tiles per exp strip


def build_program(s=S, d=D, phase_log=None):
    import concourse.bass as bass
    import concourse.mybir as mybir
    import concourse.tile as tile
    from concourse import bacc

    def mark(label):
        if phase_log is not None:
            phase_log.append((label, len(nc.inst_map)))

    f32 = mybir.dt.float32
    bf16 = mybir.dt.bfloat16
    Exp = mybir.ActivationFunctionType.Exp
    Copy = mybir.ActivationFunctionType.Copy
    add_op = mybir.AluOpType.add
    mult_op = mybir.AluOpType.mult

    n_qt = s // QTS  # q tiles per batch
    n_dkt = d // 128  # contraction tiles for projections
    n_skt = s // KTS  # k tiles per batch
    n_nt = s // QTS  # token tiles (512) for proj free dim
    n_mo = (HQ * HD) // 128  # wo contraction tiles (2)

    nc = bacc.Bacc("TRN2", num_devices=N_CORES)
    xT_d = nc.declare_dram_parameter("xT", [B, d, s], bf16, isOutput=False)
    wqkv_d = nc.declare_dram_parameter("wqkv", [d, M_PROJ], bf16, isOutput=False)
    wo_d = nc.declare_dram_parameter("wo_s", [HQ * HD, d], bf16, isOutput=False)
    cos_d = nc.declare_dram_parameter("cosb", [128, s], bf16, isOutput=False)
    sin_d = nc.declare_dram_parameter("sinb", [128, s], bf16, isOutput=False)
    tri_d = nc.declare_dram_parameter("trimask", [128, 128], f32, isOutput=False)
    tri01_d = nc.declare_dram_parameter("tri01", [128, 4, QTS], bf16, isOutput=False)
    part_d = nc.declare_dram_parameter("part", [B * s, d], bf16, isOutput=True)

    with tile.TileContext(nc) as tc:
        with (
            tc.tile_pool(name="const", bufs=1) as cpool,
            tc.tile_pool(name="big", bufs=1) as bpool,
            tc.tile_pool(name="work", bufs=3) as wpool,
            tc.tile_pool(name="estrip", bufs=5) as epool,
            tc.tile_pool(name="outp", bufs=4) as opool,
            tc.tile_pool(name="psw", bufs=3, space="PSUM") as psw,
            tc.tile_pool(name="pssc", bufs=2, space="PSUM") as pssc,
            tc.tile_pool(name="psops", bufs=1, space="PSUM") as psops,
        ):
            # ---- constants / weights ----
            cos_sb = cpool.tile([128, s], bf16)
            sin_sb = cpool.tile([128, s], bf16)
            tri_sb = cpool.tile([128, 128], f32)
            tri01_sb = cpool.tile([128, 4, QTS], bf16)
            ones_sb = cpool.tile([1, 64], f32)
            wqkv_sb = cpool.tile([128, n_dkt, M_PROJ], bf16)
            wo_sb = cpool.tile([128, n_mo, d], bf16)

            nc.sync.dma_start(cos_sb[:], cos_d[:])
            nc.sync.dma_start(sin_sb[:], sin_d[:])
            nc.sync.dma_start(tri_sb[:], tri_d[:])
            nc.sync.dma_start(tri01_sb[:], tri01_d[:])
            nc.gpsimd.memset(ones_sb[:], 1.0)
            for kt in range(n_dkt):
                nc.sync.dma_start(
                    wqkv_sb[:, kt, :], wqkv_d[kt * 128 : (kt + 1) * 128, :]
                )
            for kt in range(n_mo):
                nc.sync.dma_start(wo_sb[:, kt, :], wo_d[kt * 128 : (kt + 1) * 128, :])

            # ---- per-batch persistent tiles ----
            tiles = {}

            def load_x(b):
                xT_sb = bpool.tile([128, n_dkt, s], bf16, tag="xT")
                tiles[("x", b)] = xT_sb
                for kt in range(n_dkt):
                    nc.sync.dma_start(
                        xT_sb[:, kt, :],
                        xT_d[b, kt * 128 : (kt + 1) * 128, :],
                    )

            def proj(b):
                mark(f"b{b}_proj")
                xT_sb = tiles[("x", b)]
                QT_sb = bpool.tile([128, n_mo, s], bf16, tag="QT")
                KT2_sb = bpool.tile([128, s], bf16, tag="KT2")
                VT_sb = bpool.tile([128, s], bf16, tag="VT")
                V_sb = bpool.tile([128, n_skt, 128], bf16, tag="V")
                tiles[("QT", b)] = QT_sb
                tiles[("KT2", b)] = KT2_sb
                tiles[("V", b)] = V_sb
                # ones column / zero pad for PV lhsT
                nc.gpsimd.memset(V_sb[:, :, 64:128], 0.0)
                nc.gpsimd.memset(V_sb[:, :, 64:65], 1.0)
                for m in (2, 0, 1):  # K/V first so attention can start early
                    for n in range(n_nt):
                        nsl = slice(n * QTS, (n + 1) * QTS)
                        ps = psw.tile([128, QTS], f32, tag="w")
                        for kt in range(n_dkt):
                            nc.tensor.matmul(
                                ps[:],
                                wqkv_sb[:, kt, m * 128 : (m + 1) * 128],
                                xT_sb[:, kt, nsl],
                                start=(kt == 0),
                                stop=(kt == n_dkt - 1),
                            )
                        if m < 2:
                            # two Q heads stacked: rope all 128 rows
                            q_raw = wpool.tile([128, QTS], bf16, tag="qraw")
                            nc.scalar.activation(q_raw[:], ps[:], Copy)
                            t1 = wpool.tile([128, QTS], bf16, tag="t1")
                            t2 = wpool.tile([128, QTS], bf16, tag="t2")
                            nc.vector.tensor_tensor(
                                t1[:], q_raw[:], cos_sb[:, nsl], mult_op
                            )
                            # swap(q_raw) via cross-base copies, then * sin
                            qsw = wpool.tile([128, QTS], bf16, tag="qsw")
                            for r0, r1 in ((0, 32), (32, 0), (64, 96), (96, 64)):
                                nc.vector.tensor_copy(
                                    qsw[r0 : r0 + 32, :], q_raw[r1 : r1 + 32, :]
                                )
                            nc.vector.tensor_tensor(
                                t2[:], qsw[:], sin_sb[:, nsl], mult_op
                            )
                            nc.vector.tensor_tensor(
                                QT_sb[:, m, nsl], t1[:], t2[:], add_op
                            )
                        else:
                            # rows 0:64 = K^T (rope), rows 64:128 = V^T (copy)
                            q_raw = wpool.tile([128, QTS], bf16, tag="qraw")
                            nc.scalar.activation(q_raw[0:64, :], ps[0:64, :], Copy)
                            t1 = wpool.tile([128, QTS], bf16, tag="t1")
                            t2 = wpool.tile([128, QTS], bf16, tag="t2")
                            nc.vector.tensor_tensor(
                                t1[0:64, :], q_raw[0:64, :], cos_sb[0:64, nsl], mult_op
                            )
                            qsw = wpool.tile([128, QTS], bf16, tag="qsw")
                            for r0, r1 in ((0, 32), (32, 0)):
                                nc.vector.tensor_copy(
                                    qsw[r0 : r0 + 32, :], q_raw[r1 : r1 + 32, :]
                                )
                            nc.vector.tensor_tensor(
                                t2[0:64, :], qsw[0:64, :], sin_sb[0:64, nsl], mult_op
                            )
                            nc.vector.tensor_tensor(
                                KT2_sb[0:64, nsl], t1[0:64, :], t2[0:64, :], add_op
                            )
                            # duplicate K^T into partitions 64:128 (row-group packing)
                            nc.vector.tensor_copy(
                                KT2_sb[64:128, nsl], KT2_sb[0:64, nsl]
                            )
                            # V^T: plain cast copy into partitions 64:128
                            nc.scalar.activation(
                                VT_sb[64:128, nsl], ps[64:128, :], Copy
                            )
                    if m == 2:
                        # V^T -> V (token-major) via DMA transpose
                        mark(f"b{b}_vtr")
                        for kt in range(n_skt):
                            nc.sync.dma_start_transpose(
                                V_sb[:, kt, 0:64],
                                VT_sb[64:128, kt * KTS : (kt + 1) * KTS],
                            )
                        mark(f"b{b}_proj2")

            def attn(b):
                mark(f"b{b}_attn")
                QT_sb = tiles[("QT", b)]
                KT2_sb = tiles[("KT2", b)]
                V_sb = tiles[("V", b)]
                OT_sb = bpool.tile([128, n_mo, s], bf16, tag="OT")
                tiles[("OT", b)] = OT_sb
                pending = []

                def normalize(hb2, m2, qsl2, ops2, rt2):
                    # recip already issued (DVE); broadcast + scale into OT
                    bps = psw.tile([128, QTS], f32, tag="w")
                    nc.tensor.matmul(
                        bps[0:64, :], ones_sb[:], rt2[:], start=True, stop=True
                    )
                    bsb = wpool.tile([64, QTS], f32, tag="bsb")
                    nc.any.tensor_copy(bsb[:], bps[0:64, :])
                    nc.vector.tensor_tensor(
                        OT_sb[hb2 : hb2 + 64, m2, qsl2],
                        ops2[0:64, :],
                        bsb[:],
                        mult_op,
                    )

                for qt in range(n_qt):
                    for h in range(HQ):
                        hb = (h % 2) * 64
                        qh = QT_sb[hb : hb + 64, h // 2, :]
                        kt2 = KT2_sb[hb : hb + 64, :]
                        qsl = slice(qt * QTS, (qt + 1) * QTS)
                        n_kt = (qt + 1) * (QTS // KTS)  # k tiles needed
                        ops = psops.tile([128, QTS], f32, tag="ops")
                        for g in range(0, n_kt, GRP):
                            kts = list(range(g, min(g + GRP, n_kt)))
                            sc = pssc.tile([128, GRP * QTS], f32, tag="sc")
                            e = epool.tile([128, GRP * QTS], bf16, tag="e")
                            for j, kt in enumerate(kts):
                                nc.tensor.matmul(
                                    sc[:, j * QTS : (j + 1) * QTS],
                                    kt2[:, kt * KTS : (kt + 1) * KTS],
                                    qh[:, qsl],
                                    start=True,
                                    stop=True,
                                )
                            if g == 0 and pending:
                                # normalize the previous q-tile now; its recip
                                # had time to finish, so PE doesn't stall
                                normalize(*pending.pop())
                            nc.scalar.activation(
                                e[:, 0 : len(kts) * QTS], sc[:, 0 : len(kts) * QTS], Exp
                            )
                            for j, kt in enumerate(kts):
                                o = kt * KTS - qt * QTS
                                if o >= 0:  # diagonal tile: 0/1 mask after exp
                                    nc.vector.tensor_tensor(
                                        e[:, j * QTS : (j + 1) * QTS],
                                        e[:, j * QTS : (j + 1) * QTS],
                                        tri01_sb[:, o // KTS, :],
                                        mult_op,
                                    )
                                nc.tensor.matmul(
                                    ops[:],
                                    V_sb[:, kt, :],
                                    e[:, j * QTS : (j + 1) * QTS],
                                    start=(kt == 0),
                                    stop=(kt == n_kt - 1),
                                )
                        rt = wpool.tile([1, QTS], f32, tag="rt")
                        nc.vector.reciprocal(rt[:], ops[64:65, :])
                        pending.append((hb, h // 2, qsl, ops, rt))
                    if qt > 0:
                        wo_block(b, qt - 1)
                if pending:
                    normalize(*pending.pop())
                wo_block(b, n_qt - 1)

            def wo_block(b, qt):
                OT_sb = tiles[("OT", b)]
                for mt in range(4 * qt, 4 * qt + 4):
                    msl = slice(mt * 128, (mt + 1) * 128)
                    osb = opool.tile([128, d], bf16, tag="osb")
                    for n in range(d // QTS):
                        nsl = slice(n * QTS, (n + 1) * QTS)
                        pool = psw if n % 2 == 0 else pssc
                        ps = pool.tile([128, QTS], f32, tag="w" if n % 2 == 0 else "sc")
                        for kt in range(n_mo):
                            nc.tensor.matmul(
                                ps[:],
                                OT_sb[:, kt, msl],
                                wo_sb[:, kt, nsl],
                                start=(kt == 0),
                                stop=(kt == n_mo - 1),
                            )
                        nc.any.tensor_copy(osb[:, nsl], ps[:])
                    nc.sync.dma_start(
                        part_d[b * s + mt * 128 : b * s + (mt + 1) * 128, :],
                        osb[:],
                    )

            load_x(0)
            proj(0)
            load_x(1)  # b1 input load overlaps b0 attention (SP order)
            attn(0)
            proj(1)
            attn(1)
    mark("end")
    nc.compile()
    return nc


# ---------------- host-side sharding ----------------

_PERM = np.concatenate([np.arange(0, HD, 2), np.arange(1, HD, 2)])  # evens, odds


def make_core_inputs(x, freqs_cos, freqs_sin, wq, wk, wv, wo, s=S, d=D):
    """Build per-core input maps (list of dicts, one per core)."""
    b = x.shape[0]
    xT = np.ascontiguousarray(np.transpose(x, (0, 2, 1))).astype(BF16)  # [B, D, S]

    cosT = np.ascontiguousarray(freqs_cos.T)  # [32, S]
    sinT = np.ascontiguousarray(freqs_sin.T)
    cosb = np.tile(np.concatenate([cosT, cosT], axis=0), (2, 1)).astype(BF16)  # [128,S]
    sinb = np.tile(
        np.concatenate([-sinT, sinT], axis=0), (2, 1)
    ).astype(BF16)

    p = np.arange(128)[:, None]
    f = np.arange(128)[None, :]
    trimask = np.where(f >= p, 0.0, -1e9).astype(np.float32)
    f5 = np.arange(QTS)[None, :]
    tri01 = np.stack(
        [np.where(f5 >= o + p, 1.0, 0.0) for o in (0, 128, 256, 384)], axis=1
    ).astype(BF16)  # [128, 4, 512]

    scale = 1.0 / math.sqrt(HD)
    in_maps = []
    for c in range(N_CORES):
        wq_c = np.concatenate(
            [
                wq[:, (4 * c + h) * HD : (4 * c + h + 1) * HD][:, _PERM]
                for h in range(HQ)
            ],
            axis=1,
        ) * scale
        wk_c = wk[:, c * HD : (c + 1) * HD][:, _PERM]
        wv_c = wv[:, c * HD : (c + 1) * HD]
        wqkv = np.concatenate([wq_c, wk_c, wv_c], axis=1).astype(BF16)  # [D, 384]
        wo_c = np.ascontiguousarray(
            wo[4 * c * HD : (4 * c + HQ) * HD, :]
        ).astype(BF16)  # [256, D] — O is in original d-order (V unpermuted)
        in_maps.append(
            {
                "xT": xT,
                "wqkv": wqkv,
                "wo_s": wo_c,
                "cosb": cosb,
                "sinb": sinb,
                "trimask": trimask,
                "tri01": tri01,
            }
        )
    return in_maps


_NC_CACHE = {}


def kernel(x, freqs_cos, freqs_sin, wq, wk, wv, wo):
    from concourse.bass_utils import run_bass_kernel_spmd

    x = np.asarray(x, np.float32)
    freqs_cos = np.asarray(freqs_cos, np.float32)
    freqs_sin = np.asarray(freqs_sin, np.float32)
    wq = np.asarray(wq, np.float32)
    wk = np.asarray(wk, np.float32)
    wv = np.asarray(wv, np.float32)
    wo = np.asarray(wo, np.float32)

    if "nc" not in _NC_CACHE:
        _NC_CACHE["nc"] = build_program()
    nc = _NC_CACHE["nc"]

    in_maps = make_core_inputs(x, freqs_cos, freqs_sin, wq, wk, wv, wo)
    res = run_bass_kernel_spmd(nc, in_maps, list(range(N_CORES)))
    acc = np.zeros((B * S, D), np.float32)
    for r in res.results:
        acc += np.asarray(r["part"], np.float32)
    return acc.reshape(B, S, D).astype(BF16)



# revision 2
# speedup vs baseline: 1.1626x; 1.1626x over previous
"""Trainium2 Bass kernel for nn_Attention_78151224918608.

Dense transformer attention block: QKV proj + RoPE + GQA causal attention
+ output proj. Sharding: tensor-parallel over heads across 8 cores
(core c: Q heads 4c..4c+3, KV head c). Each core computes a partial
output (its heads through wo rows); host sums the 8 bf16 partials in
fp32 and casts to bf16.

Layout strategy (per core, per batch):
  - All matmul operands bf16; accumulation fp32 in PSUM.
  - Projections computed transposed: QKV^T[384, S] = wqkv^T @ x^T so that
    Q^T/K^T (head-dim on partitions) feed the scores matmul directly.
  - RoPE: even/odd pair interleave is folded into wq/wk/wo columns on the
    host (perm = evens-then-odds), turning the pair swap into a 32-row
    block swap done with cross-partition copies on DVE.
  - Scores computed transposed per (b,h): S^T[k,q] = K^T.T @ Q^T, so the
    softmax denominator and P@V both contract over k = partitions:
    PV lhsT = [V | ones-col] gives O^T rows 0:64 and sumexp in row 64.
  - Causal: scores/exp/PV matmuls are column-clipped to the staircase;
    diagonal 128x128 windows get a 0/1 lower-tri multiply after exp.
  - Schedule: x is DMA-streamed n-major (token-tile chunks of all 16
    d-tiles); per token tile: KV proj -> Q proj -> attention for that
    q-tile, with output-projection (wo) work for the previous q-tile
    interleaved between attention pipeline steps to keep PE fed while
    the Activation engine runs exp. b1's x load and projections overlap
    b0's attention (KT2/VT/V/OT tiles double-buffered).
"""

import sys

sys.path.insert(0, "/opt/trn_rl_repo")

import math
from collections import deque
import numpy as np
import ml_dtypes

BF16 = ml_dtypes.bfloat16

# Problem constants (hardcoded per contract).
B = 2
S = 2048
D = 2048
N_HEADS = 32
N_KV_HEADS = 8
HD = 64
N_CORES = 8
HQ = N_HEADS // N_CORES  # 4 q heads per core
M_PROJ = HQ * HD + 2 * HD  # 384: [Q0 Q1 Q2 Q3 | K | V]
QTS = 512  # q tile size (free dim)
KTS = 128  # k tile size (partitions)


def build_program(s=S, d=D, phase_log=None):
    import concourse.bass as bass
    import concourse.mybir as mybir
    import concourse.tile as tile
    from concourse import bacc

    def mark(label):
        if phase_log is not None:
            phase_log.append((label, len(nc.inst_map)))

    f32 = mybir.dt.float32
    bf16 = mybir.dt.bfloat16
    Exp = mybir.ActivationFunctionType.Exp
    Copy = mybir.ActivationFunctionType.Copy
    add_op = mybir.AluOpType.add
    mult_op = mybir.AluOpType.mult

    n_qt = s // QTS  # q tiles per batch (4)
    n_dkt = d // 128  # contraction tiles for projections (16)
    n_skt = s // KTS  # k tiles per batch (16)
    n_mo = (HQ * HD) // 128  # wo contraction tiles (2)

    nc = bacc.Bacc("TRN2", num_devices=N_CORES)
    xT_d = nc.declare_dram_parameter("xT", [B, d, s], bf16, isOutput=False)
    wqkv_d = nc.declare_dram_parameter("wqkv", [d, M_PROJ], bf16, isOutput=False)
    wo_d = nc.declare_dram_parameter("wo_s", [HQ * HD, d], bf16, isOutput=False)
    cos_d = nc.declare_dram_parameter("cosb", [128, s], bf16, isOutput=False)
    sin_d = nc.declare_dram_parameter("sinb", [128, s], bf16, isOutput=False)
    tri_d = nc.declare_dram_parameter("tri128", [128, 128], bf16, isOutput=False)
    part_d = nc.declare_dram_parameter("part", [B * s, d], bf16, isOutput=True)

    with tile.TileContext(nc) as tc:
        with (
            tc.tile_pool(name="const", bufs=1) as cpool,
            tc.tile_pool(name="big", bufs=1) as bpool,
            tc.tile_pool(name="work", bufs=3) as wpool,
            tc.tile_pool(name="estrip", bufs=6) as epool,
            tc.tile_pool(name="outp", bufs=4) as opool,
            tc.tile_pool(name="norm", bufs=4) as rpool,
            tc.tile_pool(name="pssc", bufs=2, space="PSUM") as pssc,
            tc.tile_pool(name="psops", bufs=2, space="PSUM") as psops,
            tc.tile_pool(name="psw", bufs=2, space="PSUM") as psw,
        ):
            # ---- constants / weights ----
            cos_sb = cpool.tile([128, s], bf16)
            sin_sb = cpool.tile([128, s], bf16)
            tri_sb = cpool.tile([128, 128], bf16)
            wqkv_sb = cpool.tile([128, n_dkt, M_PROJ], bf16)
            wo_sb = cpool.tile([128, n_mo, d], bf16)

            for kt in range(n_dkt):
                nc.sync.dma_start(
                    wqkv_sb[:, kt, :], wqkv_d[kt * 128 : (kt + 1) * 128, :]
                )
            nc.sync.dma_start(cos_sb[:], cos_d[:])
            nc.sync.dma_start(sin_sb[:], sin_d[:])
            nc.sync.dma_start(tri_sb[:], tri_d[:])
            for kt in range(n_mo):
                nc.sync.dma_start(wo_sb[:, kt, :], wo_d[kt * 128 : (kt + 1) * 128, :])

            tiles = {}

            def get_batch_tiles(b):
                if ("xT", b) not in tiles:
                    tiles[("xT", b)] = bpool.tile(
                        [128, n_dkt, s], bf16, tag="xT", name=f"xT{b}"
                    )
                    tiles[("QT", b)] = bpool.tile(
                        [128, n_mo, s], bf16, tag="QT", name=f"QT{b}"
                    )
                    tiles[("KT2", b)] = bpool.tile(
                        [128, s], bf16, tag="KT2", bufs=2, name=f"KT2{b}"
                    )
                    tiles[("VT", b)] = bpool.tile(
                        [128, s], bf16, tag="VT", bufs=2, name=f"VT{b}"
                    )
                    tiles[("V", b)] = bpool.tile(
                        [128, n_skt, 128], bf16, tag="V", bufs=2, name=f"V{b}"
                    )
                    tiles[("OT", b)] = bpool.tile(
                        [128, n_mo, s], bf16, tag="OT", bufs=2, name=f"OT{b}"
                    )
                return tiles

            def load_x_chunk(b, n):
                """DMA one token-tile chunk of x^T: all d-tiles, cols nsl."""
                xT_sb = get_batch_tiles(b)[("xT", b)]
                nsl = slice(n * QTS, (n + 1) * QTS)
                for kt in range(n_dkt):
                    nc.sync.dma_start(
                        xT_sb[:, kt, nsl],
                        xT_d[b, kt * 128 : (kt + 1) * 128, nsl],
                    )

            def rope_pair(dst, ps_src, rows, nsl, swaps):
                """RoPE on `rows` partitions of a psum tile into dst cols nsl."""
                r = slice(0, rows)
                q_raw = wpool.tile([128, QTS], bf16, tag="qraw")
                nc.scalar.activation(q_raw[r, :], ps_src[r, :], Copy)
                t1 = wpool.tile([128, QTS], bf16, tag="t1")
                t2 = wpool.tile([128, QTS], bf16, tag="t2")
                nc.vector.tensor_tensor(t1[r, :], q_raw[r, :], cos_sb[r, nsl], mult_op)
                qsw = wpool.tile([128, QTS], bf16, tag="qsw")
                for r0, r1 in swaps:
                    nc.vector.tensor_copy(qsw[r0 : r0 + 32, :], q_raw[r1 : r1 + 32, :])
                nc.vector.tensor_tensor(t2[r, :], qsw[r, :], sin_sb[r, nsl], mult_op)
                nc.vector.tensor_tensor(dst, t1[r, :], t2[r, :], add_op)

            def kv_proj_block(b, n):
                """K/V projection for token tile n: 16 matmuls + evac."""
                mark(f"b{b}n{n}_kv")
                bt = get_batch_tiles(b)
                xT_sb = bt[("xT", b)]
                KT2_sb = bt[("KT2", b)]
                VT_sb = bt[("VT", b)]
                V_sb = bt[("V", b)]
                nsl = slice(n * QTS, (n + 1) * QTS)
                if n == 0:
                    # ones column / zero pad for PV lhsT
                    nc.gpsimd.memset(V_sb[:, :, 64:128], 0.0)
                    nc.gpsimd.memset(V_sb[:, :, 64:65], 1.0)
                ps = psw.tile([128, QTS], f32, tag="w")
                for kt in range(n_dkt):
                    nc.tensor.matmul(
                        ps[:],
                        wqkv_sb[:, kt, 256:384],
                        xT_sb[:, kt, nsl],
                        start=(kt == 0),
                        stop=(kt == n_dkt - 1),
                    )
                # rows 0:64 = K^T (rope), rows 64:128 = V^T (copy)
                rope_pair(KT2_sb[0:64, nsl], ps, 64, nsl, ((0, 32), (32, 0)))
                # duplicate K^T into partitions 64:128 (row-group packing)
                nc.vector.tensor_copy(KT2_sb[64:128, nsl], KT2_sb[0:64, nsl])
                # V^T: plain cast copy into partitions 64:128
                nc.scalar.activation(VT_sb[64:128, nsl], ps[64:128, :], Copy)
                # V^T -> V (token-major) via DMA transpose
                for kt in range(n * 4, n * 4 + 4):
                    nc.sync.dma_start_transpose(
                        V_sb[:, kt, 0:64],
                        VT_sb[64:128, kt * KTS : (kt + 1) * KTS],
                    )

            def q_proj_block(b, n, m):
                """Q projection for head pair m (heads 2m, 2m+1), token tile n."""
                bt = get_batch_tiles(b)
                xT_sb = bt[("xT", b)]
                QT_sb = bt[("QT", b)]
                nsl = slice(n * QTS, (n + 1) * QTS)
                ps = psw.tile([128, QTS], f32, tag="w")
                for kt in range(n_dkt):
                    nc.tensor.matmul(
                        ps[:],
                        wqkv_sb[:, kt, m * 128 : (m + 1) * 128],
                        xT_sb[:, kt, nsl],
                        start=(kt == 0),
                        stop=(kt == n_dkt - 1),
                    )
                rope_pair(
                    QT_sb[:, m, nsl], ps, 128, nsl, ((0, 32), (32, 0), (64, 96), (96, 64))
                )

            # ---- wo filler machinery ----
            wo_queue = deque()

            def wo_unit(b, mt, nw):
                """One wo output tile [128 tokens, 512 d-cols]."""
                OT_sb = tiles[("OT", b)]
                msl = slice(mt * 128, (mt + 1) * 128)
                nsl = slice(nw * QTS, (nw + 1) * QTS)
                osb = tiles.get(("osb", b, mt))
                if osb is None:
                    osb = opool.tile([128, d], bf16, tag="osb", name=f"osb{b}_{mt}")
                    tiles[("osb", b, mt)] = osb
                ps = psw.tile([128, QTS], f32, tag="w")
                for kt in range(n_mo):
                    nc.tensor.matmul(
                        ps[:],
                        OT_sb[:, kt, msl],
                        wo_sb[:, kt, nsl],
                        start=(kt == 0),
                        stop=(kt == n_mo - 1),
                    )
                nc.vector.tensor_copy(osb[:, nsl], ps[:])
                if nw == d // QTS - 1:
                    nc.sync.dma_start(
                        part_d[b * s + mt * 128 : b * s + (mt + 1) * 128, :],
                        osb[:],
                    )
                    del tiles[("osb", b, mt)]

            def pop_filler(k=1):
                for _ in range(k):
                    if wo_queue:
                        b_, mt_, nw_ = wo_queue.popleft()
                        wo_unit(b_, mt_, nw_)

            def queue_wo(b, qt):
                for mt in range(4 * qt, 4 * qt + 4):
                    for nw in range(d // QTS):
                        wo_queue.append((b, mt, nw))

            # ---- attention ----
            def attn_qtile(b, qt, evac_parity):
                mark(f"b{b}_attn{qt}")
                bt = get_batch_tiles(b)
                QT_sb = bt[("QT", b)]
                KT2_sb = bt[("KT2", b)]
                V_sb = bt[("V", b)]
                OT_sb = bt[("OT", b)]
                n_kt = (qt + 1) * (QTS // KTS)  # k tiles needed
                G = n_kt // 2  # strip groups of 2 k-tiles
                qsl = slice(qt * QTS, (qt + 1) * QTS)

                def emit_scores(h, g, sc, e):
                    hb = (h % 2) * 64
                    qh = QT_sb[hb : hb + 64, h // 2, :]
                    kt2 = KT2_sb[hb : hb + 64, :]
                    los = []
                    for j in (0, 1):
                        kt = 2 * g + j
                        o = kt * KTS - qt * QTS
                        lo = max(0, o)
                        los.append(lo)
                        nc.tensor.matmul(
                            sc[:, j, lo:QTS],
                            kt2[:, kt * KTS : (kt + 1) * KTS],
                            qh[:, qt * QTS + lo : (qt + 1) * QTS],
                            start=True,
                            stop=True,
                        )
                    # exp (clipped); diagonal windows get 0/1 lower-tri mask
                    if los[0] == 0 and los[1] == 0 and 2 * g + 1 < 4 * qt:
                        nc.scalar.activation(e[:, :, :], sc[:, :, :], Exp)
                    else:
                        for j in (0, 1):
                            nc.scalar.activation(
                                e[:, j, los[j] : QTS], sc[:, j, los[j] : QTS], Exp
                            )
                    for j in (0, 1):
                        kt = 2 * g + j
                        o = kt * KTS - qt * QTS
                        if o >= 0:
                            nc.gpsimd.tensor_tensor(
                                e[:, j, o : o + KTS],
                                e[:, j, o : o + KTS],
                                tri_sb[:],
                                mult_op,
                            )
                    return los

                def emit_pv(h, g, e, los, ops):
                    for j in (0, 1):
                        kt = 2 * g + j
                        lo = los[j]
                        nc.tensor.matmul(
                            ops[:, lo:QTS],
                            V_sb[:, kt, :],
                            e[:, j, lo:QTS],
                            start=(kt == 0),
                            stop=(kt == n_kt - 1),
                        )

                for pair in (0, 1):
                    heads = (2 * pair, 2 * pair + 1)
                    ops = {}
                    pend = {}  # h -> (sc, e, los) awaiting PV
                    for h in heads:
                        ops[h] = psops.tile(
                            [128, QTS], f32, tag="ops", name=f"ops{h}"
                        )
                    for g in range(G + 1):
                        for h in heads:
                            if g < G:
                                sc = pssc.tile([128, 2, QTS], f32, tag="sc")
                                e = epool.tile([128, 2, QTS], bf16, tag="e")
                                los = emit_scores(h, g, sc, e)
                                pend[(h, g)] = (e, los)
                        for h in heads:
                            if g > 0:
                                e, los = pend.pop((h, g - 1))
                                emit_pv(h, g - 1, e, los, ops[h])
                        pop_filler(1)
                    # normalize: evacuate O^T+sumexp to SBUF, recip, broadcast,
                    # scale into OT
                    for h in heads:
                        hb = (h % 2) * 64
                        osum = rpool.tile([72, QTS], f32, tag="osum")
                        if evac_parity:
                            nc.scalar.activation(
                                osum[0:65, :], ops[h][0:65, :], Copy
                            )
                        else:
                            nc.vector.tensor_copy(osum[0:65, :], ops[h][0:65, :])
                        rt = rpool.tile([1, QTS], f32, tag="rt")
                        nc.vector.reciprocal(rt[:], osum[64:65, :])
                        bsb = rpool.tile([64, QTS], f32, tag="bsb")
                        nc.gpsimd.partition_broadcast(bsb[:], rt[:])
                        nc.vector.tensor_tensor(
                            OT_sb[hb : hb + 64, h // 2, qsl],
                            osum[0:64, :],
                            bsb[:],
                            mult_op,
                        )
                    pop_filler(1)

            # ---------------- schedule ----------------
            mark("x0_load")
            for n in range(n_qt):
                load_x_chunk(0, n)
            for b in (0, 1):
                for n in range(n_qt):
                    kv_proj_block(b, n)
                    q_proj_block(b, n, 0)
                    pop_filler(1)
                    q_proj_block(b, n, 1)
                    if b == 0:
                        load_x_chunk(1, n)
                    pop_filler(1)
                    attn_qtile(b, n, evac_parity=n % 2)
                    queue_wo(b, n)
            mark("drain")
            while wo_queue:
                pop_filler(1)
    mark("end")
    nc.compile()
    return nc


# ---------------- host-side sharding ----------------

_PERM = np.concatenate([np.arange(0, HD, 2), np.arange(1, HD, 2)])  # evens, odds


def make_core_inputs(x, freqs_cos, freqs_sin, wq, wk, wv, wo, s=S, d=D):
    """Build per-core input maps (list of dicts, one per core)."""
    xT = np.ascontiguousarray(np.transpose(x, (0, 2, 1))).astype(BF16)  # [B, D, S]

    cosT = np.ascontiguousarray(freqs_cos.T)  # [32, S]
    sinT = np.ascontiguousarray(freqs_sin.T)
    cosb = np.tile(np.concatenate([cosT, cosT], axis=0), (2, 1)).astype(BF16)  # [128,S]
    sinb = np.tile(np.concatenate([-sinT, sinT], axis=0), (2, 1)).astype(BF16)

    p = np.arange(128)[:, None]
    c = np.arange(128)[None, :]
    tri128 = np.where(c >= p, 1.0, 0.0).astype(BF16)

    scale = 1.0 / math.sqrt(HD)
    in_maps = []
    for cidx in range(N_CORES):
        wq_c = np.concatenate(
            [
                wq[:, (4 * cidx + h) * HD : (4 * cidx + h + 1) * HD][:, _PERM]
                for h in range(HQ)
            ],
            axis=1,
        ) * scale
        wk_c = wk[:, cidx * HD : (cidx + 1) * HD][:, _PERM]
        wv_c = wv[:, cidx * HD : (cidx + 1) * HD]
        wqkv = np.concatenate([wq_c, wk_c, wv_c], axis=1).astype(BF16)  # [D, 384]
        wo_c = np.ascontiguousarray(
            wo[4 * cidx * HD : (4 * cidx + HQ) * HD, :]
        ).astype(BF16)  # [256, D] — O is in original d-order (V unpermuted)
        in_maps.append(
            {
                "xT": xT,
                "wqkv": wqkv,
                "wo_s": wo_c,
                "cosb": cosb,
                "sinb": sinb,
                "tri128": tri128,
            }
        )
    return in_maps


_NC_CACHE = {}


def kernel(x, freqs_cos, freqs_sin, wq, wk, wv, wo):
    from concourse.bass_utils import run_bass_kernel_spmd

    x = np.asarray(x, np.float32)
    freqs_cos = np.asarray(freqs_cos, np.float32)
    freqs_sin = np.asarray(freqs_sin, np.float32)
    wq = np.asarray(wq, np.float32)
    wk = np.asarray(wk, np.float32)
    wv = np.asarray(wv, np.float32)
    wo = np.asarray(wo, np.float32)

    if "nc" not in _NC_CACHE:
        _NC_CACHE["nc"] = build_program()
    nc = _NC_CACHE["nc"]

    in_maps = make_core_inputs(x, freqs_cos, freqs_sin, wq, wk, wv, wo)
    res = run_bass_kernel_spmd(nc, in_maps, list(range(N_CORES)))
    acc = np.zeros((B * S, D), np.float32)
    for r in res.results:
        acc += np.asarray(r["part"], np.float32)
    return acc.reshape(B, S, D).astype(BF16)


# revision 5
# speedup vs baseline: 1.2568x; 1.0811x over previous
"""Trainium2 Bass kernel for nn_Attention_78151224918608.

Dense transformer attention block: QKV proj + RoPE + GQA causal attention
+ output proj. Sharding: tensor-parallel over heads across 8 cores
(core c: Q heads 4c..4c+3, KV head c). Each core computes a partial
output (its heads through wo rows); host sums the 8 bf16 partials in
fp32 and casts to bf16.

Layout strategy (per core, per batch):
  - All matmul operands bf16; accumulation fp32 in PSUM.
  - Projections computed transposed: QKV^T[384, S] = wqkv^T @ x^T so that
    Q^T/K^T (head-dim on partitions) feed the scores matmul directly.
  - RoPE: even/odd pair interleave is folded into wq/wk/wo columns on the
    host (perm = evens-then-odds), turning the pair swap into a 32-row
    block swap done with cross-partition copies on DVE.
  - Scores computed transposed per (b,h): S^T[k,q] = K^T.T @ Q^T, so the
    softmax denominator and P@V both contract over k = partitions:
    PV lhsT = [V | ones-col] gives O^T rows 0:64 and sumexp in row 64.
  - Causal: scores/exp/PV matmuls are column-clipped to the staircase;
    diagonal 128x128 windows get a 0/1 lower-tri multiply after exp.
  - Schedule: x is DMA-streamed n-major (token-tile chunks of all 16
    d-tiles); per token tile: KV proj -> Q proj -> attention for that
    q-tile, with output-projection (wo) work for the previous q-tile
    interleaved between attention pipeline steps to keep PE fed while
    the Activation engine runs exp. b1's x load and projections overlap
    b0's attention (KT2/VT/V/OT tiles double-buffered).
"""

import sys

sys.path.insert(0, "/opt/trn_rl_repo")

import math
from collections import deque
import numpy as np
import ml_dtypes

BF16 = ml_dtypes.bfloat16

# Problem constants (hardcoded per contract).
B = 2
S = 2048
D = 2048
N_HEADS = 32
N_KV_HEADS = 8
HD = 64
N_CORES = 8
HQ = N_HEADS // N_CORES  # 4 q heads per core
M_PROJ = HQ * HD + 2 * HD  # 384: [Q0 Q1 Q2 Q3 | K | V]
QTS = 512  # q tile size (free dim)
KTS = 128  # k tile size (partitions)


def build_program(s=S, d=D, phase_log=None):
    import concourse.bass as bass
    import concourse.mybir as mybir
    import concourse.tile as tile
    from concourse import bacc

    def mark(label):
        if phase_log is not None:
            phase_log.append((label, len(nc.inst_map)))

    f32 = mybir.dt.float32
    bf16 = mybir.dt.bfloat16
    Exp = mybir.ActivationFunctionType.Exp
    Copy = mybir.ActivationFunctionType.Copy
    add_op = mybir.AluOpType.add
    mult_op = mybir.AluOpType.mult

    n_qt = s // QTS  # q tiles per batch (4)
    n_dkt = d // 128  # contraction tiles for projections (16)
    n_skt = s // KTS  # k tiles per batch (16)
    n_mo = (HQ * HD) // 128  # wo contraction tiles (2)

    nc = bacc.Bacc("TRN2", num_devices=N_CORES)
    xT_d = nc.declare_dram_parameter("xT", [B, d, s], bf16, isOutput=False)
    wqkv_d = nc.declare_dram_parameter("wqkv", [d, M_PROJ], bf16, isOutput=False)
    wo_d = nc.declare_dram_parameter("wo_s", [HQ * HD, d], bf16, isOutput=False)
    cos_d = nc.declare_dram_parameter("cosb", [128, s], bf16, isOutput=False)
    sin_d = nc.declare_dram_parameter("sinb", [128, s], bf16, isOutput=False)
    tri_d = nc.declare_dram_parameter("tri128", [128, 128], bf16, isOutput=False)
    part_d = nc.declare_dram_parameter("part", [B * s, d], bf16, isOutput=True)

    with tile.TileContext(nc) as tc:
        with (
            tc.tile_pool(name="const", bufs=1) as cpool,
            tc.tile_pool(name="big", bufs=1) as bpool,
            tc.tile_pool(name="work", bufs=3) as wpool,
            tc.tile_pool(name="estrip", bufs=6) as epool,
            tc.tile_pool(name="outp", bufs=4) as opool,
            tc.tile_pool(name="norm", bufs=4) as rpool,
            tc.tile_pool(name="pssc", bufs=2, space="PSUM") as pssc,
            tc.tile_pool(name="psops", bufs=2, space="PSUM") as psops,
            tc.tile_pool(name="psw", bufs=2, space="PSUM") as psw,
        ):
            # ---- constants / weights ----
            cos_sb = cpool.tile([128, s], bf16)
            sin_sb = cpool.tile([128, s], bf16)
            tri_sb = cpool.tile([128, 128], bf16)
            wqkv_sb = cpool.tile([128, n_dkt, M_PROJ], bf16)
            wo_sb = cpool.tile([128, n_mo, d], bf16)

            # K/V weight columns first so the first projection can start as
            # soon as the first x chunk lands; Q columns + wo arrive behind it.
            nc.sync.dma_start(
                wqkv_sb[:, :, 256:384],
                wqkv_d[:, 256:384].rearrange("(j p) c -> p j c", p=128),
            )

            tiles = {}

            def get_batch_tiles(b):
                if ("xT", b) not in tiles:
                    tiles[("xT", b)] = bpool.tile(
                        [128, n_dkt, s], bf16, tag="xT", name=f"xT{b}"
                    )
                    tiles[("QT", b)] = bpool.tile(
                        [128, n_mo, s], bf16, tag="QT", name=f"QT{b}"
                    )
                    tiles[("KT2", b)] = bpool.tile(
                        [128, s], bf16, tag="KT2", bufs=2, name=f"KT2{b}"
                    )
                    tiles[("VT", b)] = bpool.tile(
                        [128, s], bf16, tag="VT", bufs=2, name=f"VT{b}"
                    )
                    tiles[("V", b)] = bpool.tile(
                        [128, n_skt, 128], bf16, tag="V", bufs=2, name=f"V{b}"
                    )
                    tiles[("OT", b)] = bpool.tile(
                        [128, n_mo, s], bf16, tag="OT", bufs=2, name=f"OT{b}"
                    )
                return tiles

            def load_x_chunk(b, n):
                """DMA one token-tile chunk of x^T: all d-tiles, cols nsl."""
                xT_sb = get_batch_tiles(b)[("xT", b)]
                nsl = slice(n * QTS, (n + 1) * QTS)
                nc.sync.dma_start(
                    xT_sb[:, :, nsl],
                    xT_d[b, :, nsl].rearrange("(j p) c -> p j c", p=128),
                )

            def rope_pair(dst, ps_src, rows, nsl, swaps):
                """RoPE on `rows` partitions of a psum tile into dst cols nsl."""
                r = slice(0, rows)
                q_raw = wpool.tile([128, QTS], bf16, tag="qraw")
                nc.scalar.activation(q_raw[r, :], ps_src[r, :], Copy)
                t1 = wpool.tile([128, QTS], bf16, tag="t1")
                t2 = wpool.tile([128, QTS], bf16, tag="t2")
                nc.vector.tensor_tensor(t1[r, :], q_raw[r, :], cos_sb[r, nsl], mult_op)
                qsw = wpool.tile([128, QTS], bf16, tag="qsw")
                for r0, r1 in swaps:
                    nc.vector.tensor_copy(qsw[r0 : r0 + 32, :], q_raw[r1 : r1 + 32, :])
                nc.vector.tensor_tensor(t2[r, :], qsw[r, :], sin_sb[r, nsl], mult_op)
                nc.vector.tensor_tensor(dst, t1[r, :], t2[r, :], add_op)

            def kv_proj_block(b, n):
                """K/V projection for token tile n: 16 matmuls + evac."""
                mark(f"b{b}n{n}_kv")
                bt = get_batch_tiles(b)
                xT_sb = bt[("xT", b)]
                KT2_sb = bt[("KT2", b)]
                VT_sb = bt[("VT", b)]
                V_sb = bt[("V", b)]
                nsl = slice(n * QTS, (n + 1) * QTS)
                if n == 0:
                    # ones column / zero pad for PV lhsT
                    nc.gpsimd.memset(V_sb[:, :, 64:128], 0.0)
                    nc.gpsimd.memset(V_sb[:, :, 64:65], 1.0)
                ps = psw.tile([128, QTS], f32, tag="w")
                for kt in range(n_dkt):
                    nc.tensor.matmul(
                        ps[:],
                        wqkv_sb[:, kt, 256:384],
                        xT_sb[:, kt, nsl],
                        start=(kt == 0),
                        stop=(kt == n_dkt - 1),
                    )
                # rows 0:64 = K^T (rope), rows 64:128 = V^T (copy)
                rope_pair(KT2_sb[0:64, nsl], ps, 64, nsl, ((0, 32), (32, 0)))
                # duplicate K^T into partitions 64:128 (row-group packing)
                nc.vector.tensor_copy(KT2_sb[64:128, nsl], KT2_sb[0:64, nsl])
                # V^T: plain cast copy into partitions 64:128
                nc.scalar.activation(VT_sb[64:128, nsl], ps[64:128, :], Copy)
                # V^T -> V (token-major) via DMA transpose
                for kt in range(n * 4, n * 4 + 4):
                    nc.sync.dma_start_transpose(
                        V_sb[:, kt, 0:64],
                        VT_sb[64:128, kt * KTS : (kt + 1) * KTS],
                    )

            def q_proj_block(b, n, m):
                """Q projection for head pair m (heads 2m, 2m+1), token tile n."""
                bt = get_batch_tiles(b)
                xT_sb = bt[("xT", b)]
                QT_sb = bt[("QT", b)]
                nsl = slice(n * QTS, (n + 1) * QTS)
                ps = psw.tile([128, QTS], f32, tag="w")
                for kt in range(n_dkt):
                    nc.tensor.matmul(
                        ps[:],
                        wqkv_sb[:, kt, m * 128 : (m + 1) * 128],
                        xT_sb[:, kt, nsl],
                        start=(kt == 0),
                        stop=(kt == n_dkt - 1),
                    )
                rope_pair(
                    QT_sb[:, m, nsl], ps, 128, nsl, ((0, 32), (32, 0), (64, 96), (96, 64))
                )

            # ---- wo filler machinery ----
            wo_queue = deque()

            def wo_unit(b, mt, nw):
                """One wo output tile [128 tokens, 512 d-cols]."""
                OT_sb = tiles[("OT", b)]
                msl = slice(mt * 128, (mt + 1) * 128)
                nsl = slice(nw * QTS, (nw + 1) * QTS)
                osb = tiles.get(("osb", b, mt))
                if osb is None:
                    osb = opool.tile([128, d], bf16, tag="osb", name=f"osb{b}_{mt}")
                    tiles[("osb", b, mt)] = osb
                ps = psw.tile([128, QTS], f32, tag="w")
                for kt in range(n_mo):
                    nc.tensor.matmul(
                        ps[:],
                        OT_sb[:, kt, msl],
                        wo_sb[:, kt, nsl],
                        start=(kt == 0),
                        stop=(kt == n_mo - 1),
                    )
                nc.vector.tensor_copy(osb[:, nsl], ps[:])
                if nw == d // QTS - 1:
                    nc.sync.dma_start(
                        part_d[b * s + mt * 128 : b * s + (mt + 1) * 128, :],
                        osb[:],
                    )
                    del tiles[("osb", b, mt)]

            def pop_filler(k=1):
                for _ in range(k):
                    if wo_queue:
                        b_, mt_, nw_ = wo_queue.popleft()
                        wo_unit(b_, mt_, nw_)

            def queue_wo(b, qt):
                for mt in range(4 * qt, 4 * qt + 4):
                    for nw in range(d // QTS):
                        wo_queue.append((b, mt, nw))

            # ---- attention ----
            def attn_qtile(b, qt, evac_parity):
                mark(f"b{b}_attn{qt}")
                bt = get_batch_tiles(b)
                QT_sb = bt[("QT", b)]
                KT2_sb = bt[("KT2", b)]
                V_sb = bt[("V", b)]
                OT_sb = bt[("OT", b)]
                n_kt = (qt + 1) * (QTS // KTS)  # k tiles needed
                G = n_kt // 2  # strip groups of 2 k-tiles
                qsl = slice(qt * QTS, (qt + 1) * QTS)

                def emit_scores(h, g, sc, e):
                    hb = (h % 2) * 64
                    qh = QT_sb[hb : hb + 64, h // 2, :]
                    kt2 = KT2_sb[hb : hb + 64, :]
                    los = []
                    for j in (0, 1):
                        kt = 2 * g + j
                        o = kt * KTS - qt * QTS
                        lo = max(0, o)
                        los.append(lo)
                        nc.tensor.matmul(
                            sc[:, j, lo:QTS],
                            kt2[:, kt * KTS : (kt + 1) * KTS],
                            qh[:, qt * QTS + lo : (qt + 1) * QTS],
                            start=True,
                            stop=True,
                        )
                    # exp (clipped); diagonal windows get 0/1 lower-tri mask
                    if los[0] == 0 and los[1] == 0 and 2 * g + 1 < 4 * qt:
                        nc.scalar.activation(e[:, :, :], sc[:, :, :], Exp)
                    else:
                        for j in (0, 1):
                            nc.scalar.activation(
                                e[:, j, los[j] : QTS], sc[:, j, los[j] : QTS], Exp
                            )
                    for j in (0, 1):
                        kt = 2 * g + j
                        o = kt * KTS - qt * QTS
                        if o >= 0:
                            nc.gpsimd.tensor_tensor(
                                e[:, j, o : o + KTS],
                                e[:, j, o : o + KTS],
                                tri_sb[:],
                                mult_op,
                            )
                    return los

                def emit_pv(h, g, e, los, ops):
                    for j in (0, 1):
                        kt = 2 * g + j
                        lo = los[j]
                        nc.tensor.matmul(
                            ops[:, lo:QTS],
                            V_sb[:, kt, :],
                            e[:, j, lo:QTS],
                            start=(kt == 0),
                            stop=(kt == n_kt - 1),
                        )

                for pair in (0, 1):
                    heads = (2 * pair, 2 * pair + 1)
                    ops = {}
                    pend = {}  # h -> (sc, e, los) awaiting PV
                    for h in heads:
                        ops[h] = psops.tile(
                            [128, QTS], f32, tag="ops", name=f"ops{h}"
                        )
                    for g in range(G + 1):
                        for h in heads:
                            if g < G:
                                sc = pssc.tile([128, 2, QTS], f32, tag="sc")
                                e = epool.tile([128, 2, QTS], bf16, tag="e")
                                los = emit_scores(h, g, sc, e)
                                pend[(h, g)] = (e, los)
                        for h in heads:
                            if g > 0:
                                e, los = pend.pop((h, g - 1))
                                emit_pv(h, g - 1, e, los, ops[h])
                        pop_filler(1)
                    # normalize: evacuate O^T+sumexp to SBUF, recip, broadcast,
                    # scale into OT
                    for h in heads:
                        hb = (h % 2) * 64
                        osum = rpool.tile([72, QTS], f32, tag="osum")
                        if evac_parity:
                            nc.scalar.activation(
                                osum[0:65, :], ops[h][0:65, :], Copy
                            )
                        else:
                            nc.vector.tensor_copy(osum[0:65, :], ops[h][0:65, :])
                        rt = rpool.tile([1, QTS], f32, tag="rt")
                        nc.vector.reciprocal(rt[:], osum[64:65, :])
                        bsb = rpool.tile([64, QTS], f32, tag="bsb")
                        nc.gpsimd.partition_broadcast(bsb[:], rt[:])
                        nc.vector.tensor_tensor(
                            OT_sb[hb : hb + 64, h // 2, qsl],
                            osum[0:64, :],
                            bsb[:],
                            mult_op,
                        )
                    pop_filler(1)

            # ---------------- schedule ----------------
            mark("x0_load")
            load_x_chunk(0, 0)
            nc.sync.dma_start(cos_sb[:], cos_d[:])
            nc.sync.dma_start(sin_sb[:], sin_d[:])
            nc.sync.dma_start(tri_sb[:], tri_d[:])
            nc.sync.dma_start(
                wqkv_sb[:, :, 0:256],
                wqkv_d[:, 0:256].rearrange("(j p) c -> p j c", p=128),
            )
            for n in range(1, n_qt):
                load_x_chunk(0, n)
            nc.sync.dma_start(
                wo_sb[:, :, :], wo_d[:, :].rearrange("(j p) c -> p j c", p=128)
            )
            for b in (0, 1):
                for n in range(n_qt):
                    kv_proj_block(b, n)
                    q_proj_block(b, n, 0)
                    pop_filler(1)
                    q_proj_block(b, n, 1)
                    if b == 0:
                        load_x_chunk(1, n)
                    pop_filler(1)
                    attn_qtile(b, n, evac_parity=n % 2)
                    queue_wo(b, n)
            mark("drain")
            while wo_queue:
                pop_filler(1)
    mark("end")
    nc.compile()
    return nc


# ---------------- host-side sharding ----------------

_PERM = np.concatenate([np.arange(0, HD, 2), np.arange(1, HD, 2)])  # evens, odds


def make_core_inputs(x, freqs_cos, freqs_sin, wq, wk, wv, wo, s=S, d=D):
    """Build per-core input maps (list of dicts, one per core)."""
    xT = np.ascontiguousarray(np.transpose(x, (0, 2, 1))).astype(BF16)  # [B, D, S]

    cosT = np.ascontiguousarray(freqs_cos.T)  # [32, S]
    sinT = np.ascontiguousarray(freqs_sin.T)
    cosb = np.tile(np.concatenate([cosT, cosT], axis=0), (2, 1)).astype(BF16)  # [128,S]
    sinb = np.tile(np.concatenate([-sinT, sinT], axis=0), (2, 1)).astype(BF16)

    p = np.arange(128)[:, None]
    c = np.arange(128)[None, :]
    tri128 = np.where(c >= p, 1.0, 0.0).astype(BF16)

    scale = 1.0 / math.sqrt(HD)
    in_maps = []
    for cidx in range(N_CORES):
        wq_c = np.concatenate(
            [
                wq[:, (4 * cidx + h) * HD : (4 * cidx + h + 1) * HD][:, _PERM]
                for h in range(HQ)
            ],
            axis=1,
        ) * scale
        wk_c = wk[:, cidx * HD : (cidx + 1) * HD][:, _PERM]
        wv_c = wv[:, cidx * HD : (cidx + 1) * HD]
        wqkv = np.concatenate([wq_c, wk_c, wv_c], axis=1).astype(BF16)  # [D, 384]
        wo_c = np.ascontiguousarray(
            wo[4 * cidx * HD : (4 * cidx + HQ) * HD, :]
        ).astype(BF16)  # [256, D] — O is in original d-order (V unpermuted)
        in_maps.append(
            {
                "xT": xT,
                "wqkv": wqkv,
                "wo_s": wo_c,
                "cosb": cosb,
                "sinb": sinb,
                "tri128": tri128,
            }
        )
    return in_maps


_NC_CACHE = {}


def kernel(x, freqs_cos, freqs_sin, wq, wk, wv, wo):
    from concourse.bass_utils import run_bass_kernel_spmd

    x = np.asarray(x, np.float32)
    freqs_cos = np.asarray(freqs_cos, np.float32)
    freqs_sin = np.asarray(freqs_sin, np.float32)
    wq = np.asarray(wq, np.float32)
    wk = np.asarray(wk, np.float32)
    wv = np.asarray(wv, np.float32)
    wo = np.asarray(wo, np.float32)

    if "nc" not in _NC_CACHE:
        _NC_CACHE["nc"] = build_program()
    nc = _NC_CACHE["nc"]

    in_maps = make_core_inputs(x, freqs_cos, freqs_sin, wq, wk, wv, wo)
    res = run_bass_kernel_spmd(nc, in_maps, list(range(N_CORES)))
    acc = np.zeros((B * S, D), np.float32)
    for r in res.results:
        acc += np.asarray(r["part"], np.float32)
    return acc.reshape(B, S, D).astype(BF16)
